# revision 17
# baseline (speedup 1.0000x reference)
"""Trainium2 Bass kernel for nn_ChildHAggregation (gnn_message_passing).

Computation per sample:
  x = [hl, hr]                                        (1024)
  q_t = (h_t @ qU + qU_b) * su_q + sb_q   t in {l,r}  su/sb from xh
  k_t = (h_t @ kU + kU_b) * su_k + sb_k
  2x2 softmax attention over the two tokens -> per-sample probs
  x += scores @ [hl, hr]; layernorm(ddof=1) with alpha/beta
  out = (x @ hU + hU_b) * (xh @ hWu + hWu_b) + (xh @ hWb + hWb_b)
      + (xw @ lU + lU_b) * (xh @ lWu + lWu_b) + (xh @ lWb + lWb_b)

v2 strategy (pure data-parallel over 8 cores, batch 32768 -> 4096/core):
  - batch-major tiles [128 x features]; PE-transposed inputs are the
    stationary operands so outputs land batch-major in PSUM.
  - transposes use a bf16 identity as the moving operand (1 cyc/row on
    the PE vs 2 for fp32) with fp32r-bitcast data.
  - hdT = (hl - hr)^T computed directly from the hl/hr transpose PSUMs
    with one subtract (no separate hd transpose).
  - 2-token softmax == sigmoid of score differences; the post-attention
    x' = [a0*hl + b0*hr, a1*hl + b1*hr] is built EXPLICITLY (2 scaled
    copies + 2 scalar_tensor_tensor), transposed, and pushed through
    hU as a single 1024-contraction matmul.  Row sums ride free on the
    x'-build accum; sum-of-squares are 2 accum passes -> exact
    layernorm stats with no folded quadratic algebra.
  - all "U-bias * hyper" folds from v1 kept: qWb' = qWb + qWu*diag(qU_b),
    WC = hWb' + lWb', cs = colsum(alpha*hU), bh folded into WC/cb.
  - elementwise work split across Vector/Scalar/GpSimd to keep every
    engine under the tensor-engine roofline.
"""

import os
from contextlib import ExitStack

import numpy as np

import concourse.bacc as bacc
import concourse.bass as bass
import concourse.mybir as mybir
import concourse.tile as tile
from concourse.bass_utils import run_bass_kernel_spmd
from concourse.masks import make_identity

N_CORES = 8
B_FULL = 32768
HALF = 512
DIM = 1024
P = 128
EPS = 1e-6
INV_SQRT_HALF = 1.0 / float(np.sqrt(np.float32(HALF)))

f32 = mybir.dt.float32
f32r = mybir.dt.float32r
bf16 = mybir.dt.bfloat16

AX = mybir.AxisListType
ALU = mybir.AluOpType
ACTF = mybir.ActivationFunctionType

W512 = ["qU", "kU", "qWu", "qWb", "kWu", "kWb", "hWu", "hWb", "lWu", "lWb"]


def _mm(ap, mm_dt):
    """Bitcast an fp32 AP to the matmul dtype (f32 or f32r; same bits)."""
    if mm_dt == f32:
        return ap
    return ap.bitcast(mm_dt)


def build_nc(b_loc, mm_dt=f32r, stage=99):
    """Build the per-core Bass program for a local batch of b_loc rows."""
    n_tiles = b_loc // P
    assert n_tiles * P == b_loc

    nc = bacc.Bacc("TRN2", target_bir_lowering=False, debug=False,
                   num_devices=1)

    # identity dtype for PE transposes: f32r streams 1.5 cyc/row (vs 2.0
    # for fp32; bf16 would be 1.0 but NCC rejects mixed 32/16-bit matmul)
    tid_env = os.environ.get("KERNEL_TID", "f32r")

    # ---- DRAM I/O (names match setup_inputs) ----
    d = {}
    d["hl"] = nc.dram_tensor("hl", [b_loc, HALF], f32, kind="ExternalInput").ap()
    d["hr"] = nc.dram_tensor("hr", [b_loc, HALF], f32, kind="ExternalInput").ap()
    d["xw"] = nc.dram_tensor("xw", [b_loc, DIM], f32, kind="ExternalInput").ap()
    d["xh"] = nc.dram_tensor("xh", [b_loc, HALF], f32, kind="ExternalInput").ap()
    for w in W512:
        d[w + "_w"] = nc.dram_tensor(w + "_w", [HALF, HALF], f32,
                                     kind="ExternalInput").ap()
        d[w + "_b"] = nc.dram_tensor(w + "_b", [HALF], f32,
                                     kind="ExternalInput").ap()
    for w in ["hU", "lU"]:
        d[w + "_w"] = nc.dram_tensor(w + "_w", [DIM, HALF], f32,
                                     kind="ExternalInput").ap()
        d[w + "_b"] = nc.dram_tensor(w + "_b", [HALF], f32,
                                     kind="ExternalInput").ap()
    d["alpha"] = nc.dram_tensor("alpha", [DIM], f32, kind="ExternalInput").ap()
    d["beta"] = nc.dram_tensor("beta", [DIM], f32, kind="ExternalInput").ap()
    out_d = nc.dram_tensor("out", [b_loc, HALF], f32, kind="ExternalOutput").ap()

    with tile.TileContext(nc) as tc, ExitStack() as ctx:
        # ================= persistent pools =================
        wts = ctx.enter_context(tc.tile_pool(name="wts", bufs=1))
        biasp = ctx.enter_context(tc.tile_pool(name="biasp", bufs=1))

        wsb = {}
        for w in ["qU", "kU", "qWu", "kWu", "hWu", "lWu", "qWb", "hWb"]:
            wsb[w] = wts.tile([P, 4, HALF], f32, name=f"w_{w}")
        for w in ["hU", "lU"]:
            wsb[w] = wts.tile([P, 8, HALF], f32, name=f"w_{w}")
        alpha_sb = wts.tile([P, 8], f32)
        nc.sync.dma_start(alpha_sb, d["alpha"].rearrange("(c p) -> p c", p=P))
        beta_sb = wts.tile([P, 8], f32)
        nc.sync.dma_start(beta_sb, d["beta"].rearrange("(c p) -> p c", p=P))
        ident = wts.tile([P, P], f32)
        make_identity(nc, ident)
        if tid_env == "f32r":
            # f32r consumers need a rounding producer: copy through an
            # f32r-typed write (1.0/0.0 round exactly)
            identt_t = wts.tile([P, P], f32, name="identr")
            identt = _mm(identt_t, f32r)
            nc.vector.tensor_copy(identt, ident)
        else:
            identt = ident

        def tp(ps_ap, src_ap):
            """PE transpose; f32r operands stream 1.5 cyc/row vs 2.0 fp32."""
            if tid_env == "fp32":
                nc.tensor.transpose(ps_ap, src_ap, ident)
            else:
                nc.tensor.transpose(_mm(ps_ap, f32r), _mm(src_ap, f32r), identt)

        def dma_in(dst, src_ap):
            """DMA inputs as f32r so transposes can consume them directly."""
            if tid_env == "fp32":
                nc.sync.dma_start(dst, src_ap)
            else:
                nc.sync.dma_start(_mm(dst, f32r), _mm(src_ap, f32r))

        # persistent broadcast [P, 512] bias tiles
        bc = {}
        for nm in ["qWu_b", "kWu_b", "hWu_b", "lWu_b", "qb", "cb", "cs"]:
            bc[nm] = biasp.tile([P, HALF], f32, name=f"bc_{nm}")

        # ================= input pool + prefetch =================
        inp = ctx.enter_context(tc.tile_pool(name="inp", bufs=2))
        prefetched = {}
        for i in range(min(2, n_tiles)):
            for nm, wdt in (("hl", HALF), ("hr", HALF), ("xh", HALF),
                            ("xw", DIM)):
                t = inp.tile([P, wdt], f32, tag=nm, name=f"pre_{nm}_{i}")
                dma_in(t, d[nm][bass.ts(i, P), :])
                prefetched[(i, nm)] = t

        # ---------------- one-time setup (same folds as v1) ----------------
        with tc.tile_pool(name="setup", bufs=1) as sp, \
                tc.tile_pool(name="psum_setup", bufs=2, space="PSUM") as psum_setup:

            def bias_row(nm):
                r = sp.tile([1, HALF], f32, tag="row", bufs=2, name=f"row_{nm}")
                nc.sync.dma_start(r, d[nm][None, :])
                return r

            def bcast(dst, row_ap):
                nc.gpsimd.partition_broadcast(dst, row_ap)

            def tmp_bc(nm, row_ap):
                t = sp.tile([P, HALF], f32, tag="tbc", bufs=5, name=f"tbc_{nm}")
                bcast(t, row_ap)
                return t

            for nm in ["qWu_b", "kWu_b", "hWu_b", "lWu_b"]:
                bcast(bc[nm], bias_row(nm))

            qUb_bc = tmp_bc("qU_b", bias_row("qU_b"))
            lUb_bc = tmp_bc("lU_b", bias_row("lU_b"))

            ftmp = sp.tile([P, HALF], f32, tag="ftmp")
            bcast(bc["qb"], bias_row("qWb_b"))
            nc.vector.tensor_mul(ftmp, qUb_bc, bc["qWu_b"])
            nc.vector.tensor_add(bc["qb"], bc["qb"], ftmp)

            def wtemp(w, nch):
                t = sp.tile([P, nch, HALF], f32, tag=f"wtmp{nch}",
                            bufs=(3 if nch == 4 else 1), name=f"wtmp_{w}")
                rr = d[w + "_w"].rearrange("(c p) o -> p c o", p=P)
                for c in range(nch):
                    nc.sync.dma_start(t[:, c, :], rr[:, c, :])
                return t

            wt_tmps = {}
            for w in ["qU", "kU", "qWu", "kWu"]:
                t = wt_tmps[w] = wtemp(w, 4)
                for c in range(4):
                    nc.vector.tensor_copy(_mm(wsb[w][:, c, :], mm_dt), t[:, c, :])
            hU_tmp = wtemp("hU", 8)
            for w in ["hWu", "lWu"]:
                t = wt_tmps[w] = wtemp(w, 4)
                for c in range(4):
                    nc.vector.tensor_copy(_mm(wsb[w][:, c, :], mm_dt), t[:, c, :])

            bh_ps = psum_setup.tile([1, HALF], f32)
            cs_ps = psum_setup.tile([1, HALF], f32)
            for c in range(8):
                nc.tensor.matmul(bh_ps, beta_sb[:, c:c + 1], hU_tmp[:, c, :],
                                 start=(c == 0), stop=(c == 7))
            for c in range(8):
                nc.tensor.matmul(cs_ps, alpha_sb[:, c:c + 1], hU_tmp[:, c, :],
                                 start=(c == 0), stop=(c == 7))
            bh_row = sp.tile([1, HALF], f32, tag="row", bufs=2)
            nc.vector.tensor_add(bh_row, bh_ps, bias_row("hU_b"))
            cs_row = sp.tile([1, HALF], f32, tag="row", bufs=2)
            nc.vector.tensor_copy(cs_row, cs_ps)
            bcast(bc["cs"], cs_row)
            bh_bc = tmp_bc("bh", bh_row)
            for c in range(8):
                nc.vector.tensor_scalar_mul(_mm(wsb["hU"][:, c, :], mm_dt),
                                            hU_tmp[:, c, :],
                                            alpha_sb[:, c:c + 1])

            lU_tmp = wtemp("lU", 8)
            for c in range(8):
                nc.vector.tensor_copy(_mm(wsb["lU"][:, c, :], mm_dt),
                                      lU_tmp[:, c, :])

            # cb = (hWb_b + bh*hWu_b) + (lWb_b + lU_b*lWu_b)
            bcast(bc["cb"], bias_row("hWb_b"))
            nc.vector.tensor_mul(ftmp, bh_bc, bc["hWu_b"])
            nc.vector.tensor_add(bc["cb"], bc["cb"], ftmp)
            lWbb_bc = tmp_bc("lWb_b", bias_row("lWb_b"))
            nc.vector.tensor_add(bc["cb"], bc["cb"], lWbb_bc)
            nc.vector.tensor_mul(ftmp, lUb_bc, bc["lWu_b"])
            nc.vector.tensor_add(bc["cb"], bc["cb"], ftmp)

            # qWb' = qWb + qWu*diag(qU_b) ; WC = hWb + hWu*diag(bh)
            #                                   + lWb + lWu*diag(lU_b)
            qWb_tmp = wtemp("qWb", 4)
            for c in range(4):
                nc.vector.tensor_mul(ftmp, wsb["qWu"][:, c, :], qUb_bc)
                nc.vector.tensor_add(_mm(wsb["qWb"][:, c, :], mm_dt),
                                     qWb_tmp[:, c, :], ftmp)
            hWb_tmp = wtemp("hWb", 4)
            lWb_tmp = wtemp("lWb", 4)
            for c in range(4):
                nc.vector.tensor_mul(ftmp, wsb["hWu"][:, c, :], bh_bc)
                nc.vector.tensor_add(hWb_tmp[:, c, :], hWb_tmp[:, c, :], ftmp)
                nc.vector.tensor_add(hWb_tmp[:, c, :], hWb_tmp[:, c, :],
                                     lWb_tmp[:, c, :])
                nc.vector.tensor_mul(ftmp, wsb["lWu"][:, c, :], lUb_bc)
                nc.vector.tensor_add(_mm(wsb["hWb"][:, c, :], mm_dt),
                                     hWb_tmp[:, c, :], ftmp)

        # ================= main loop pools =================
        tsp = ctx.enter_context(tc.tile_pool(name="tsp", bufs=2))
        pha = ctx.enter_context(tc.tile_pool(name="pha", bufs=1))
        scr = ctx.enter_context(tc.tile_pool(name="scr", bufs=3))
        tinyp = ctx.enter_context(tc.tile_pool(name="tinyp", bufs=2))
        phd = ctx.enter_context(tc.tile_pool(name="phd", bufs=1))
        outp = ctx.enter_context(tc.tile_pool(name="outp", bufs=2))
        tp_ps = ctx.enter_context(tc.tile_pool(name="tp_ps", bufs=2, space="PSUM"))
        mm_ps = ctx.enter_context(tc.tile_pool(name="mm_ps", bufs=6, space="PSUM"))

        for i in range(n_tiles):
            rs = bass.ts(i, P)
            # ---- loads ----
            if (i, "hl") in prefetched:
                hl_t = prefetched.pop((i, "hl"))
                hr_t = prefetched.pop((i, "hr"))
                xh_t = prefetched.pop((i, "xh"))
                xw_t = prefetched.pop((i, "xw"))
            else:
                hl_t = inp.tile([P, HALF], f32, tag="hl")
                dma_in(hl_t, d["hl"][rs, :])
                hr_t = inp.tile([P, HALF], f32, tag="hr")
                dma_in(hr_t, d["hr"][rs, :])
                xh_t = inp.tile([P, HALF], f32, tag="xh")
                dma_in(xh_t, d["xh"][rs, :])
                xw_t = inp.tile([P, DIM], f32, tag="xw")
                dma_in(xw_t, d["xw"][rs, :])

            def stage_out(src_ap):
                ot = outp.tile([P, HALF], f32, tag="out_t", name=f"out_stage_{i}")
                nc.vector.tensor_copy(ot, src_ap)
                nc.sync.dma_start(out_d[rs, :], ot)

            if stage == 1:
                stage_out(hl_t)
                continue

            # ---- PE transposes of the raw inputs ----
            def tgroup(src, g, tg):
                ps = tp_ps.tile([P, 4 * P], f32, tag="tp",
                                name=f"tps_{tg}_{g}_{i}")
                for c in range(4):
                    tp(ps[:, c * P:(c + 1) * P],
                       src[:, (g * 4 + c) * P:(g * 4 + c + 1) * P])
                return ps

            ps_hl = tgroup(hl_t, 0, "hl")
            ps_hr = tgroup(hr_t, 0, "hr")
            hlT = tsp.tile([P, HALF], f32, tag="ThL")
            nc.scalar.copy(_mm(hlT, mm_dt), ps_hl)
            hrT = tsp.tile([P, HALF], f32, tag="ThR")
            nc.scalar.copy(_mm(hrT, mm_dt), ps_hr)
            hdT = tsp.tile([P, HALF], f32, tag="ThD")
            nc.gpsimd.tensor_sub(_mm(hdT, mm_dt), hlT, hrT)

            ps_xh = tgroup(xh_t, 0, "xh")
            xhT = tsp.tile([P, HALF], f32, tag="TxH")
            nc.scalar.copy(_mm(xhT, mm_dt), ps_xh)

            # ---- phase A matmuls ----
            def unit(tag):
                return mm_ps.tile([P, HALF], f32, tag="mm", name=f"ps_{tag}_{i}")

            SUq, SBq, TU = unit("SUq"), unit("SBq"), unit("TU")
            for c in range(4):
                lhs = _mm(xhT[:, bass.ts(c, P)], mm_dt)
                st, sp_ = (c == 0), (c == 3)
                nc.tensor.matmul(SUq, lhs, _mm(wsb["qWu"][:, c, :], mm_dt), start=st, stop=sp_)
                nc.tensor.matmul(SBq, lhs, _mm(wsb["qWb"][:, c, :], mm_dt), start=st, stop=sp_)
                nc.tensor.matmul(TU, lhs, _mm(wsb["kWu"][:, c, :], mm_dt), start=st, stop=sp_)
            A_l = unit("A_l")
            for c in range(4):
                nc.tensor.matmul(A_l, _mm(hlT[:, bass.ts(c, P)], mm_dt),
                                 _mm(wsb["qU"][:, c, :], mm_dt),
                                 start=(c == 0), stop=(c == 3))
            A_r = unit("A_r")
            for c in range(4):
                nc.tensor.matmul(A_r, _mm(hrT[:, bass.ts(c, P)], mm_dt),
                                 _mm(wsb["qU"][:, c, :], mm_dt),
                                 start=(c == 0), stop=(c == 3))
            CD = unit("CD")
            for c in range(4):
                nc.tensor.matmul(CD, _mm(hdT[:, bass.ts(c, P)], mm_dt),
                                 _mm(wsb["kU"][:, c, :], mm_dt),
                                 start=(c == 0), stop=(c == 3))

            # ---- xw transposes + phase D xh/xw matmuls (keep PE fed) ----
            ps_xw0 = tgroup(xw_t, 0, "xw0")
            xwT = tsp.tile([P, DIM], f32, tag="TxW")
            nc.scalar.copy(_mm(xwT[:, :HALF], mm_dt), ps_xw0)
            ps_xw1 = tgroup(xw_t, 1, "xw1")
            nc.scalar.copy(_mm(xwT[:, HALF:], mm_dt), ps_xw1)

            LUp = unit("LU")
            for c in range(8):
                nc.tensor.matmul(LUp, _mm(xwT[:, bass.ts(c, P)], mm_dt),
                                 _mm(wsb["lU"][:, c, :], mm_dt),
                                 start=(c == 0), stop=(c == 7))
            HSU, LSU, SBC = unit("HSU"), unit("LSU"), unit("SBC")
            for c in range(4):
                lhs = _mm(xhT[:, bass.ts(c, P)], mm_dt)
                st, sp_ = (c == 0), (c == 3)
                nc.tensor.matmul(HSU, lhs, _mm(wsb["hWu"][:, c, :], mm_dt), start=st, stop=sp_)
                nc.tensor.matmul(LSU, lhs, _mm(wsb["lWu"][:, c, :], mm_dt), start=st, stop=sp_)
                nc.tensor.matmul(SBC, lhs, _mm(wsb["hWb"][:, c, :], mm_dt), start=st, stop=sp_)

            # ---- phase A elementwise (score-difference trick) ----
            su = pha.tile([P, HALF], f32, tag="su")
            nc.vector.tensor_add(su, SUq, bc["qWu_b"])
            tu = pha.tile([P, HALF], f32, tag="tu")
            nc.vector.tensor_add(tu, TU, bc["kWu_b"])
            sbq = pha.tile([P, HALF], f32, tag="sbq")
            nc.vector.tensor_add(sbq, SBq, bc["qb"])

            if stage == 3:
                stage_out(su)
                continue

            dk = pha.tile([P, HALF], f32, tag="dk")
            nc.vector.tensor_mul(dk, CD, tu)
            u = pha.tile([P, HALF], f32, tag="u")
            nc.gpsimd.tensor_mul(u, su, dk)

            stats = tinyp.tile([P, 4], f32, tag="stats")
            for j, (aa, bb) in enumerate([(sbq, dk), (A_l, u), (A_r, u)]):
                sd = scr.tile([P, HALF], f32, tag="scr", name=f"scr_dot{j}_{i}")
                nc.vector.scalar_tensor_tensor(
                    sd, aa, 0.0, bb, ALU.bypass, ALU.mult,
                    accum_out=stats[:, j:j + 1])

            # ---- 2-way softmax via sigmoid ----
            diffs = tinyp.tile([P, 2], f32, tag="diffs")
            nc.vector.tensor_add(diffs, stats[:, 1:3],
                                 stats[:, 0:1].broadcast_to([P, 2]))
            probs = tinyp.tile([P, 2], f32, tag="probs")
            nc.scalar.activation(probs, diffs, ACTF.Sigmoid, scale=INV_SQRT_HALF)
            a0 = tinyp.tile([P, 1], f32, tag="a0")
            nc.scalar.activation(a0, probs[:, 0:1], ACTF.Copy, bias=1.0)
            b0 = tinyp.tile([P, 1], f32, tag="b0")
            nc.scalar.activation(b0, probs[:, 0:1], ACTF.Copy, scale=-1.0, bias=1.0)
            a1 = probs[:, 1:2]
            b1 = tinyp.tile([P, 1], f32, tag="b1")
            nc.scalar.activation(b1, probs[:, 1:2], ACTF.Copy, scale=-1.0, bias=2.0)

            if stage == 4:
                stage_out(u)
                continue

            # ---- explicit post-attention x' = [x0, x1] + row stats ----
            s0 = tinyp.tile([P, 1], f32, tag="s0")
            s1 = tinyp.tile([P, 1], f32, tag="s1")
            q0 = tinyp.tile([P, 1], f32, tag="q0")
            q1 = tinyp.tile([P, 1], f32, tag="q1")
            tmp0 = scr.tile([P, HALF], f32, tag="scr", name=f"scr_t0_{i}")
            nc.scalar.activation(tmp0, hr_t, ACTF.Copy, scale=b0)
            x0 = tsp.tile([P, HALF], f32, tag="x0", bufs=1)
            nc.vector.scalar_tensor_tensor(_mm(x0, mm_dt), hl_t, a0, tmp0,
                                           ALU.mult, ALU.add, accum_out=s0)
            tmp1 = scr.tile([P, HALF], f32, tag="scr", name=f"scr_t1_{i}")
            nc.scalar.activation(tmp1, hr_t, ACTF.Copy, scale=b1)
            x1 = tsp.tile([P, HALF], f32, tag="x1", bufs=1)
            nc.vector.scalar_tensor_tensor(_mm(x1, mm_dt), hl_t, a1, tmp1,
                                           ALU.mult, ALU.add, accum_out=s1)
            sq0 = scr.tile([P, HALF], f32, tag="scr", name=f"scr_q0_{i}")
            nc.vector.scalar_tensor_tensor(sq0, x0, 0.0, x0, ALU.bypass,
                                           ALU.mult, accum_out=q0)
            sq1 = scr.tile([P, HALF], f32, tag="scr", name=f"scr_q1_{i}")
            nc.vector.scalar_tensor_tensor(sq1, x1, 0.0, x1, ALU.bypass,
                                           ALU.mult, accum_out=q1)

            # ---- layernorm stats ----
            sumx = tinyp.tile([P, 1], f32, tag="sumx")
            nc.vector.tensor_add(sumx, s0, s1)
            ssqs = tinyp.tile([P, 1], f32, tag="ssqs")
            nc.vector.tensor_add(ssqs, q0, q1)
            mean = tinyp.tile([P, 1], f32, tag="mean")
            nc.vector.tensor_scalar_mul(mean, sumx, 1.0 / DIM)
            mean_n = tinyp.tile([P, 1], f32, tag="mean_n")
            nc.vector.tensor_scalar_mul(mean_n, sumx, -1.0 / DIM)
            varp = tinyp.tile([P, 1], f32, tag="varp")
            nc.vector.scalar_tensor_tensor(varp, sumx, mean_n, ssqs,
                                           ALU.mult, ALU.add)
            stde = tinyp.tile([P, 1], f32, tag="stde")
            nc.scalar.activation(stde, varp, ACTF.Sqrt, scale=1.0 / (DIM - 1))
            nstd = tinyp.tile([P, 1], f32, tag="nstd")
            nc.scalar.activation(nstd, stde, ACTF.Copy, scale=-1.0, bias=-EPS)
            nrinv = tinyp.tile([P, 1], f32, tag="nrinv")
            nc.vector.reciprocal(nrinv, nstd)

            # ---- x' transposes + HU matmul (1024-contraction) ----
            ps_x0 = tgroup(x0, 0, "x0")
            x0T = tsp.tile([P, HALF], f32, tag="Tx0", bufs=1)
            nc.scalar.copy(_mm(x0T, mm_dt), ps_x0)
            ps_x1 = tgroup(x1, 0, "x1")
            x1T = tsp.tile([P, HALF], f32, tag="Tx1", bufs=1)
            nc.scalar.copy(_mm(x1T, mm_dt), ps_x1)

            HU = unit("HU")
            for c in range(4):
                nc.tensor.matmul(HU, _mm(x0T[:, bass.ts(c, P)], mm_dt),
                                 _mm(wsb["hU"][:, c, :], mm_dt),
                                 start=(c == 0), stop=False)
            for c in range(4):
                nc.tensor.matmul(HU, _mm(x1T[:, bass.ts(c, P)], mm_dt),
                                 _mm(wsb["hU"][:, 4 + c, :], mm_dt),
                                 start=False, stop=(c == 3))

            if stage == 5:
                stage_out(dk)
                continue

            # ---- final combine ----
            su_h = phd.tile([P, HALF], f32, tag="su_h")
            nc.vector.tensor_add(su_h, HSU, bc["hWu_b"])
            su_l = phd.tile([P, HALF], f32, tag="su_l")
            nc.vector.tensor_add(su_l, LSU, bc["lWu_b"])
            sbc = phd.tile([P, HALF], f32, tag="sbc")
            nc.vector.tensor_add(sbc, SBC, bc["cb"])

            # t5p = cs*mean - HU ; v1 = (t5p * -rinv) * su_h
            t5p = phd.tile([P, HALF], f32, tag="t5p")
            nc.vector.scalar_tensor_tensor(t5p, bc["cs"], mean, HU,
                                           ALU.mult, ALU.subtract)
            v1 = phd.tile([P, HALF], f32, tag="v1")
            nc.vector.scalar_tensor_tensor(v1, t5p, nrinv, su_h,
                                           ALU.mult, ALU.mult)
            w1 = phd.tile([P, HALF], f32, tag="w1")
            nc.vector.tensor_mul(w1, LUp, su_l)
            o1 = phd.tile([P, HALF], f32, tag="o1")
            nc.gpsimd.tensor_add(o1, v1, sbc)
            out_t = outp.tile([P, HALF], f32, tag="out_t")
            nc.gpsimd.tensor_add(out_t, o1, w1)

            nc.sync.dma_start(out_d[rs, :], out_t)

    nc.compile()
    return nc


_NC_CACHE = {}


def _get_nc(b_loc, mm_dt):
    key = (b_loc, str(mm_dt))
    if key not in _NC_CACHE:
        _NC_CACHE[key] = build_nc(b_loc, mm_dt)
    return _NC_CACHE[key]


def kernel(**inputs):
    mm_dt = f32r if os.environ.get("KERNEL_MM_DT", "f32r") == "f32r" else f32
    b = inputs["hl"].shape[0]
    n_cores = N_CORES
    b_loc = b // n_cores
    nc = _get_nc(b_loc, mm_dt)

    sharded = {"hl", "hr", "xw", "xh"}
    in_maps = []
    for i in range(n_cores):
        m = {}
        for k, v in inputs.items():
            v = np.ascontiguousarray(np.asarray(v, dtype=np.float32))
            if k in sharded:
                m[k] = v[i * b_loc:(i + 1) * b_loc]
            else:
                m[k] = v
        in_maps.append(m)

    res = run_bass_kernel_spmd(nc, in_maps, core_ids=list(range(n_cores)))
    return np.concatenate([r["out"] for r in res.results], axis=0)
